# revision 23
# baseline (speedup 1.0000x reference)
"""GCN block (GCNConv + GraphNorm + ReLU + global_max_pool) on 8 Trainium2 cores.

Strategy (matches the "assign whole graphs to devices" sharding hint):
- Host: partition the 64 graphs into 8 contiguous node-balanced groups, one per
  core.  Within a core, each graph's nodes are padded to a multiple of 128 so
  every 128-node block belongs to exactly one graph.  Edges (plus self-loops)
  are bucketed by destination block and cut into 128-edge chunks.  All of this
  is index-only preprocessing; feature compute happens on device.
- Device (one SPMD Bass program on 8 NeuronCores, no collectives):
    per block: indirect-DMA gather of source rows (fp16), build the scatter
    one-hot with one fused DVE tensor_scalar (iota == dloc) * norm, accumulate
    aggT = sum_chunks XgT @ OneHot on the PE in PSUM, then out = aggT.T @ W + b,
    GraphNorm statistics via node->graph one-hot matmuls accumulated in PSUM,
    then h = out * rstd*gamma + (beta - alpha*mean*rstd*gamma), ReLU,
    PE-transpose + reduce_max per block for the pooling, DMA h out.
- Host: unshard h; fold per-block maxes into per-graph flat.
"""

import sys

if "/opt/trn_rl_repo" not in sys.path:
    sys.path.insert(0, "/opt/trn_rl_repo")

from contextlib import ExitStack
from dataclasses import dataclass

import numpy as np

import concourse.bacc as bacc
import concourse.bass as bass
import concourse.tile as tile
from concourse import mybir
from concourse._compat import get_trn_type
from concourse.library_config import mlp

F16 = mybir.dt.bfloat16
NPF16 = mybir.dt.np(mybir.dt.bfloat16)
F32 = mybir.dt.float32
I16 = mybir.dt.int16
I32 = mybir.dt.int32

NCORES = 8
P = 128
GPB = 4  # blocks per indirect-gather group
EPS = 1e-5
NUM_GRAPHS = 64


@dataclass
class Dims:
    n_rows: int  # rows of the (shared) fp16 node-feature table
    NB: int  # 128-node blocks per core
    CPQ: int  # 128-edge chunks per (group, parity class)
    GP: int  # graph slots per core (last slot = pad graph)

    @property
    def NGRP(self):
        return self.NB // GPB

    @property
    def K(self):
        # chunk slots per group: 4 parity classes x CPQ
        return 4 * self.CPQ

    @property
    def NCH(self):
        return self.NGRP * self.K


@dataclass
class Meta:
    dims: Dims
    x16: np.ndarray  # [n_rows, 128] f16
    srcidx: np.ndarray  # [NCORES, 128, NGRP*K*8] i16 gather indices (wrap-16, rows 0:32)
    ndv: np.ndarray  # [NCORES, 128, 2*NCH] f32: dloc cols [0:NCH], norm cols [NCH:2NCH]
    mg: np.ndarray  # [NCORES, 128, NB*GP] f16
    mgt: np.ndarray  # [NCORES, GP, NB*128] f16
    invcnt: np.ndarray  # [NCORES, GP, 1] f32
    w16: np.ndarray  # [128, 128] f16
    bias16: np.ndarray  # [1, 128] f16
    alpha_gp: np.ndarray  # [GP, 128] f32
    gnw_gp: np.ndarray  # [GP, 128] f32
    gnb_gp: np.ndarray  # [GP, 128] f32
    # unshard info
    core_graphs: list  # per core: list of (global_graph, count, pad_start_slot)
    node_perm: np.ndarray  # [N] global slot index per node (core-major)


def _split_graphs(pblocks: np.ndarray, ncores: int) -> list[int]:
    """Split graphs (given per-graph block counts) into ncores contiguous parts
    minimizing the max blocks per part. Returns boundaries s[0..ncores]."""
    G = len(pblocks)
    cum = np.concatenate([[0], np.cumsum(pblocks)])

    def feasible(B):
        s, parts = 0, 0
        while s < G:
            e = int(np.searchsorted(cum, cum[s] + B, side="right")) - 1
            if e <= s:
                return None
            parts += 1
            s = e
            if parts > ncores:
                return None
        return parts

    lo, hi = int(pblocks.max()), int(cum[-1])
    while lo < hi:
        mid = (lo + hi) // 2
        if feasible(mid):
            hi = mid
        else:
            lo = mid + 1
    B = lo
    bounds = [0]
    s = 0
    for c in range(ncores):
        remaining = ncores - c - 1
        e = int(np.searchsorted(cum, cum[s] + B, side="right")) - 1
        e = max(e, s + 1) if s < G else s
        # don't leave more graphs than remaining cores can take (trivially fine)
        e = min(e, G - 0)
        if remaining == 0:
            e = G
        bounds.append(e)
        s = e
    assert bounds[-1] == G, bounds
    return bounds


def preprocess(inputs, edge_index, batch, edge_weight, W, b, gn_weight, gn_bias, gn_alpha) -> Meta:
    inputs = np.asarray(inputs)
    N = inputs.shape[0]
    row = np.asarray(edge_index[0], dtype=np.int64)
    col = np.asarray(edge_index[1], dtype=np.int64)
    w = np.asarray(edge_weight, dtype=np.float32)
    batch = np.asarray(batch, dtype=np.int64)
    G = NUM_GRAPHS

    # --- GCN symmetric norm (with self-loops), f32 like the reference ---
    deg = np.bincount(col, weights=w.astype(np.float64), minlength=N).astype(np.float32)
    deg += 1.0  # self-loop weight
    dis = (1.0 / np.sqrt(deg)).astype(np.float32)
    norm_e = (dis[row] * w * dis[col]).astype(np.float32)
    norm_self = (dis * dis).astype(np.float32)

    # --- graph partitioning: contiguous groups of whole graphs ---
    counts = np.bincount(batch, minlength=G).astype(np.int64)
    gstart = np.concatenate([[0], np.cumsum(counts)])  # node range per graph
    pblocks = (counts + P - 1) // P  # blocks per graph (0 for empty)
    bounds = _split_graphs(np.maximum(pblocks, 0), NCORES)

    NB = 0
    core_graphs = []
    core_of_graph = np.zeros(G, np.int64)
    for c in range(NCORES):
        gs, ge = bounds[c], bounds[c + 1]
        core_of_graph[gs:ge] = c
        lst = []
        off = 0
        for g in range(gs, ge):
            lst.append((g, int(counts[g]), off * P))
            off += int(pblocks[g])
        core_graphs.append(lst)
        NB = max(NB, off)
    NB = ((NB + GPB - 1) // GPB) * GPB
    GP = max(len(lst) for lst in core_graphs) + 1

    # global node -> (core, local slot)
    slot_of_node = np.empty(N, np.int64)
    core_of_node = core_of_graph[batch]
    for c in range(NCORES):
        for g, cnt, ps in core_graphs[c]:
            s0 = gstart[g]
            slot_of_node[s0 : s0 + cnt] = ps + np.arange(cnt)

    # --- per-core edge buckets ---
    # combine real edges and self-loops
    all_src = np.concatenate([row, np.arange(N, dtype=np.int64)])
    all_dst = np.concatenate([col, np.arange(N, dtype=np.int64)])
    all_val = np.concatenate([norm_e, norm_self])
    dst_core = core_of_node[all_dst]
    dst_slot = slot_of_node[all_dst]

    # bucket edges by (group of 4 blocks, src parity class); sort by (grp, q)
    GSZ = GPB * P  # 512 dests per group
    NGRP = NB // GPB
    CPQ = 1
    per_core = []
    for c in range(NCORES):
        m = dst_core == c
        dsl = dst_slot[m]
        srcs = all_src[m]
        vals = all_val[m]
        grp = dsl // GSZ
        q = (srcs & 3).astype(np.int64)
        key = grp * 4 + q
        order = np.argsort(key, kind="stable")
        dsl, srcs, vals, key = dsl[order], srcs[order], vals[order], key[order]
        cnt_key = np.bincount(key, minlength=NGRP * 4)
        starts = np.concatenate([[0], np.cumsum(cnt_key)])
        pos = np.arange(len(dsl)) - starts[key]
        CPQ = max(CPQ, int((pos >> 7).max()) + 1)
        per_core.append((dsl, srcs, vals, key, pos))

    dims = Dims(n_rows=N, NB=NB, CPQ=CPQ, GP=GP)
    K = dims.K
    NCH = dims.NCH

    srcidx = np.zeros((NCORES, P, dims.NGRP * K * 8), np.int16)
    ndv = np.zeros((NCORES, P, 2 * NCH), np.float32)
    mg = np.zeros((NCORES, P, NB * GP), NPF16)
    mgt = np.zeros((NCORES, GP, NB * P), NPF16)
    invcnt = np.ones((NCORES, GP, 1), np.float32)

    for c in range(NCORES):
        dsl, srcs, vals, key, pos = per_core[c]
        grp, q = key // 4, key % 4
        cq = pos >> 7  # chunk within (grp, q)
        lane = pos & 127
        dloc = (dsl % GSZ).astype(np.float32)
        # global chunk slot
        jg = grp * K + q * CPQ + cq
        ndv[c, lane, jg] = dloc
        ndv[c, lane, NCH + jg] = vals
        # gather index: slot j within the (grp, q) gather = cq*128 + lane,
        # wrapped to [j % 16, j // 16]; HW (queue 0) reads partitions 16..31,
        # CoreSim reads 0..15 -> fill both.
        j = cq * P + lane
        colbase = (grp * 4 + q) * CPQ * 8
        i16v = (srcs >> 2).astype(np.int16)
        for m in range(5):
            srcidx[c, 16 * m + (j % 16), colbase + (j // 16)] = i16v

        # node-slot -> graph-slot map
        gsl = np.full(NB * P, GP - 1, np.int64)
        for gl, (g, cnt, ps) in enumerate(core_graphs[c]):
            gsl[ps : ps + cnt] = gl
            invcnt[c, gl, 0] = 1.0 / max(cnt, 1)
        slots = np.arange(NB * P)
        mg[c, slots & 127, (slots >> 7) * GP + gsl] = 1.0
        # mgt: pad-node columns stay all-zero so h_pad = relu(0*out + 0) = 0,
        # which never exceeds a real (relu'd, >= 0) per-graph max.
        real = gsl < GP - 1
        mgt[c, gsl[real], slots[real]] = 1.0
        # pad-graph slot: zero stats (its h/flat are discarded; keeps var >= 0)
        invcnt[c, GP - 1, 0] = 0.0

    alpha = np.asarray(gn_alpha, np.float32).reshape(1, P)
    gnw = np.asarray(gn_weight, np.float32).reshape(1, P)
    gnb = np.asarray(gn_bias, np.float32).reshape(1, P)

    return Meta(
        dims=dims,
        x16=np.ascontiguousarray(inputs.astype(NPF16)),
        srcidx=srcidx,
        ndv=ndv,
        mg=mg,
        mgt=mgt,
        invcnt=invcnt,
        w16=np.asarray(W, np.float32).astype(NPF16),
        bias16=np.asarray(b, np.float32).reshape(1, P).astype(NPF16),
        alpha_gp=np.repeat(alpha, GP, axis=0),
        gnw_gp=np.repeat(gnw, GP, axis=0),
        gnb_gp=np.repeat(gnb, GP, axis=0),
        core_graphs=core_graphs,
        node_perm=slot_of_node,
    )


def _legalize_waits(nc: bass.Bass) -> None:
    """This toolchain's walrus encodes at most ONE sync wait per instruction;
    Tile can emit several.  (1) Drop waits that same-engine program order
    already guarantees (Tile is not transitively minimal); (2) hoist remaining
    extras into standalone InstEventSemaphore instructions just before the
    consumer on the same engine."""
    updaters: dict[int, set] = {}
    for f in nc.m.functions:
        for bb in f.blocks:
            for ins in bb.instructions:
                si = ins.sync_info
                if not si or not si.on_update:
                    continue
                is_async = "DMA" in type(ins).__name__ or "Collective" in type(ins).__name__
                for u in si.on_update:
                    updaters.setdefault(u.id, set()).add((ins.engine, is_async))

    counter = 0
    for f in nc.m.functions:
        for bb in f.blocks:
            snapshot = list(bb.instructions)
            insert_at = 0
            live = bb.instructions
            for ins in snapshot:
                idx = None
                si = ins.sync_info
                if si and si.on_wait and len(si.on_wait) > 1:
                    keep = []
                    for w in si.on_wait:
                        ups = updaters.get(w.id)
                        if (
                            ups
                            and ins.engine != mybir.EngineType.PE
                            and all(
                                eng == ins.engine and not a for eng, a in ups
                            )
                        ):
                            continue  # same-engine in-order execution covers it
                        keep.append(w)
                    if not keep:
                        keep = [si.on_wait[-1]]
                    extras, keep = keep[:-1], keep[-1:]
                    while si.on_wait:
                        si.on_wait.pop()
                    si.on_wait.append(keep[0])
                    if extras:
                        idx = next(
                            i for i, x in enumerate(live) if x.name == ins.name
                        )
                        for w in extras:
                            counter += 1
                            ev = mybir.InstEventSemaphore(
                                name=f"I-lglw-{counter}", ins=[], outs=[]
                            )
                            ev.engine = ins.engine
                            evsi = ev.sync_info
                            if evsi is None:
                                ev.sync_info = mybir.SyncInfo(on_wait=[w], on_update=[])
                            else:
                                evsi.on_wait.append(w)
                            nc.register_instruction(ev, overwrite=True)
                            live.insert(idx, ev)
                            idx += 1


def build_program(dims: Dims) -> bass.Bass:
    NB, CPQ, GP, K, NGRP = dims.NB, dims.CPQ, dims.GP, dims.K, dims.NGRP
    NCH = dims.NCH
    GSZ = GPB * P
    nc = bacc.Bacc(get_trn_type() or "TRN2", num_swdge_queues=1)

    x16 = nc.dram_tensor("x16", [dims.n_rows, P], F16, kind="ExternalInput")
    srcidx = nc.dram_tensor("srcidx", [P, NGRP * K * 8], I16, kind="ExternalInput")
    ndv = nc.dram_tensor("ndv", [P, 2 * NCH], F32, kind="ExternalInput")
    mg = nc.dram_tensor("mg", [P, NB * GP], F16, kind="ExternalInput")
    mgt = nc.dram_tensor("mgt", [GP, NB * P], F16, kind="ExternalInput")
    invcnt = nc.dram_tensor("invcnt", [GP, 1], F32, kind="ExternalInput")
    w16 = nc.dram_tensor("w16", [P, P], F16, kind="ExternalInput")
    bias16 = nc.dram_tensor("bias16", [1, P], F16, kind="ExternalInput")
    alpha_gp = nc.dram_tensor("alpha_gp", [GP, P], F32, kind="ExternalInput")
    gnw_gp = nc.dram_tensor("gnw_gp", [GP, P], F32, kind="ExternalInput")
    gnb_gp = nc.dram_tensor("gnb_gp", [GP, P], F32, kind="ExternalInput")
    iota512 = nc.dram_tensor("iota512", [P, GSZ], I16, kind="ExternalInput")
    identf = nc.dram_tensor("identf", [P, P], F32, kind="ExternalInput")

    h_out = nc.dram_tensor("h_out", [NB * P, P], F32, kind="ExternalOutput")
    bmax_out = nc.dram_tensor("bmax_out", [P, NB], F32, kind="ExternalOutput")

    # strided parity views of the node table: class q = rows q, q+4, q+8, ...
    xviews = []
    for q in range(4):
        ap = bass.AP(tensor=x16, offset=q * P, ap=[[4 * P, dims.n_rows // 4], [1, P]])
        xviews.append(ap)

    with tile.TileContext(nc) as tc, ExitStack() as ctx:
        const = ctx.enter_context(tc.tile_pool(name="const", bufs=1))
        meta = ctx.enter_context(tc.tile_pool(name="meta", bufs=1))
        gpool = ctx.enter_context(tc.tile_pool(name="gather", bufs=3))
        ohp = ctx.enter_context(tc.tile_pool(name="onehot", bufs=4))
        work = ctx.enter_context(tc.tile_pool(name="work", bufs=3))
        outall = ctx.enter_context(tc.tile_pool(name="outall", bufs=NB))
        hpool = ctx.enter_context(tc.tile_pool(name="hpool", bufs=3))
        psA = ctx.enter_context(tc.tile_pool(name="psA", bufs=2, space="PSUM"))
        psO = ctx.enter_context(tc.tile_pool(name="psO", bufs=2, space="PSUM"))
        psS = ctx.enter_context(tc.tile_pool(name="psS", bufs=1, space="PSUM"))
        psB = ctx.enter_context(tc.tile_pool(name="psB", bufs=1, space="PSUM"))
        psT = ctx.enter_context(tc.tile_pool(name="psT", bufs=1, space="PSUM"))

        # gpsimd runs ONLY the library load + dma_gathers (custom ucode safety)
        nc.gpsimd.load_library(mlp)

        # ---- constants (all host-shipped; no gpsimd compute) ----
        iota_sb = const.tile([P, GSZ], I16)
        nc.sync.dma_start(iota_sb[:], iota512[:])
        ident = const.tile([P, P], F32)
        nc.sync.dma_start(ident[:], identf[:])
        ones1 = const.tile([1, P], F16)
        nc.vector.memset(ones1[:], 1.0)
        eps_t = const.tile([GP, 1], F32)
        nc.vector.memset(eps_t[:], EPS)

        w_sb = const.tile([P, P], F16)
        nc.sync.dma_start(w_sb[:], w16[:])
        b_sb = const.tile([1, P], F16)
        nc.sync.dma_start(b_sb[:], bias16[:])
        invc_sb = const.tile([GP, 1], F32)
        nc.sync.dma_start(invc_sb[:], invcnt[:])
        alpha_sb = const.tile([GP, P], F32)
        nc.sync.dma_start(alpha_sb[:], alpha_gp[:])
        gnw_sb = const.tile([GP, P], F32)
        nc.sync.dma_start(gnw_sb[:], gnw_gp[:])
        gnb_sb = const.tile([GP, P], F32)
        nc.sync.dma_start(gnb_sb[:], gnb_gp[:])

        # ---- resident metadata ----
        srcidx_sb = meta.tile([P, NGRP * K * 8], I16)
        nc.sync.dma_start(srcidx_sb[:], srcidx[:])
        nd_sb = meta.tile([P, 2 * NCH], F32)
        nc.sync.dma_start(nd_sb[:], ndv[:])
        mg_sb = meta.tile([P, NB * GP], F16)
        nc.sync.dma_start(mg_sb[:], mg[:])
        mgt_sb = meta.tile([GP, NB * P], F16)
        nc.sync.dma_start(mgt_sb[:], mgt[:])

        # ---- persistent accumulators ----
        sums_t = psS.tile([GP, P], F32, tag="sums")
        sumsq_t = psS.tile([GP, P], F32, tag="sumsq")
        sums_ps = sums_t[:]
        sumsq_ps = sumsq_t[:]
        bmax_sb = meta.tile([P, NB], F32, tag="bmax")

        out_tiles = []

        # ---- phase A/B: gather + aggregate + linear + stats ----
        for grp in range(NGRP):
            xg = gpool.tile([P, K, P], F16)
            for q in range(4):
                colbase = (grp * 4 + q) * CPQ * 8
                nc.gpsimd.dma_gather(
                    out_ap=xg[:, q * CPQ : (q + 1) * CPQ, :],
                    in_ap=xviews[q],
                    idxs_ap=srcidx_sb[:, colbase : colbase + CPQ * 8],
                    num_idxs=CPQ * P,
                    num_idxs_reg=CPQ * P,
                    elem_size=P,
                    elem_step=4 * P,
                    queue_num=0,
                )
            aggT = psA.tile([P, GSZ], F32)
            for j in range(K):
                jg = grp * K + j
                oh = ohp.tile([P, GSZ], F16, tag="oh")
                nc.vector.tensor_scalar(
                    out=oh[:],
                    in0=iota_sb[:],
                    scalar1=nd_sb[:, jg : jg + 1],
                    scalar2=nd_sb[:, NCH + jg : NCH + jg + 1],
                    op0=mybir.AluOpType.is_equal,
                    op1=mybir.AluOpType.mult,
                )
                nc.tensor.matmul(
                    out=aggT[:],
                    lhsT=xg[:, j, :],
                    rhs=oh[:],
                    start=(j == 0),
                    stop=(j == K - 1),
                )
            aggT_sb = work.tile([P, GSZ], F16, tag="aggT")
            nc.scalar.copy(aggT_sb[:], aggT[:])
            for bb in range(GPB):
                b_ = grp * GPB + bb
                outp = psO.tile([P, P], F32)
                nc.tensor.matmul(
                    outp[:], lhsT=aggT_sb[:, bb * P : (bb + 1) * P], rhs=w_sb[:],
                    start=True, stop=False,
                )
                nc.tensor.matmul(outp[:], lhsT=ones1[:], rhs=b_sb[:], start=False, stop=True)
                out_sb = outall.tile([P, P], F16, tag="out")
                nc.scalar.copy(out_sb[:], outp[:])
                out_tiles.append(out_sb)
                sq_sb = work.tile([P, P], F16, tag="sq")
                nc.scalar.square(sq_sb[:], outp[:])
                nc.tensor.matmul(
                    out=sums_ps,
                    lhsT=mg_sb[:, b_ * GP : (b_ + 1) * GP],
                    rhs=out_sb[:],
                    start=(b_ == 0),
                    stop=(b_ == NB - 1),
                )
                nc.tensor.matmul(
                    out=sumsq_ps,
                    lhsT=mg_sb[:, b_ * GP : (b_ + 1) * GP],
                    rhs=sq_sb[:],
                    start=(b_ == 0),
                    stop=(b_ == NB - 1),
                )

        # ---- phase C: finalize GraphNorm stats ([GP, 128] tiles) ----
        mu = work.tile([GP, P], F32, tag="mu")
        nc.vector.tensor_scalar(
            out=mu[:], in0=sums_ps, scalar1=invc_sb[:, 0:1], scalar2=None,
            op0=mybir.AluOpType.mult,
        )
        esq = work.tile([GP, P], F32, tag="esq")
        nc.vector.tensor_scalar(
            out=esq[:], in0=sumsq_ps, scalar1=invc_sb[:, 0:1], scalar2=None,
            op0=mybir.AluOpType.mult,
        )
        am = work.tile([GP, P], F32, tag="am")
        nc.vector.tensor_tensor(out=am[:], in0=mu[:], in1=alpha_sb[:], op=mybir.AluOpType.mult)
        t2 = work.tile([GP, P], F32, tag="t2")
        nc.vector.tensor_scalar(
            out=t2[:], in0=mu[:], scalar1=2.0, scalar2=None, op0=mybir.AluOpType.mult
        )
        nc.vector.tensor_tensor(out=t2[:], in0=t2[:], in1=am[:], op=mybir.AluOpType.subtract)
        nc.vector.tensor_tensor(out=t2[:], in0=t2[:], in1=am[:], op=mybir.AluOpType.mult)
        var = work.tile([GP, P], F32, tag="var")
        nc.vector.tensor_tensor(out=var[:], in0=esq[:], in1=t2[:], op=mybir.AluOpType.subtract)
        sd = work.tile([GP, P], F32, tag="sd")
        nc.scalar.activation(
            out=sd[:], in_=var[:], func=mybir.ActivationFunctionType.Sqrt,
            bias=eps_t[:, 0:1], scale=1.0,
        )
        rstd = work.tile([GP, P], F32, tag="rstd")
        nc.vector.reciprocal(out=rstd[:], in_=sd[:])
        rg = work.tile([GP, P], F32, tag="rg")
        nc.vector.tensor_tensor(out=rg[:], in0=rstd[:], in1=gnw_sb[:], op=mybir.AluOpType.mult)
        q_t = work.tile([GP, P], F32, tag="q_t")
        nc.vector.tensor_tensor(out=q_t[:], in0=am[:], in1=rg[:], op=mybir.AluOpType.mult)
        nc.vector.tensor_tensor(out=q_t[:], in0=gnb_sb[:], in1=q_t[:], op=mybir.AluOpType.subtract)
        combo = const.tile([GP, 2 * P], F16)
        nc.scalar.copy(combo[:, 0:P], rg[:])
        nc.scalar.copy(combo[:, P : 2 * P], q_t[:])

        # ---- phase D: normalize + relu + write + block max ----
        for b_ in range(NB):
            rq = psB.tile([P, 2 * P], F32, tag="rq")
            nc.tensor.matmul(
                out=rq[:],
                lhsT=mgt_sb[:, b_ * P : (b_ + 1) * P],
                rhs=combo[:],
                start=True,
                stop=True,
            )
            hpre = hpool.tile([P, P], F32, tag="hpre")
            nc.vector.tensor_tensor(
                out=hpre[:], in0=out_tiles[b_][:], in1=rq[:, 0:P], op=mybir.AluOpType.mult
            )
            nc.vector.tensor_tensor(
                out=hpre[:], in0=hpre[:], in1=rq[:, P : 2 * P], op=mybir.AluOpType.add
            )
            hsb = hpool.tile([P, P], F32, tag="hsb")
            nc.scalar.activation(
                out=hsb[:], in_=hpre[:], func=mybir.ActivationFunctionType.Relu
            )
            nc.sync.dma_start(h_out[b_ * P : (b_ + 1) * P, :], hsb[:])
            htp = psT.tile([P, P], F32, tag="htp")
            nc.tensor.transpose(htp[:], hsb[:], ident[:])
            nc.vector.reduce_max(bmax_sb[:, b_ : b_ + 1], htp[:], axis=mybir.AxisListType.X)

        nc.sync.dma_start(bmax_out[:], bmax_sb[:])

    _legalize_waits(nc)
    nc.compile()
    return nc


def _install_ntff_hook_shim():
    """The agent image's ``antenv`` lacks ``axon_hooks``; recreate it so
    ``run_bass_kernel_spmd(trace=True)`` can capture NTFF profiles."""
    import sys as _sys
    import types

    if "antenv.axon_hooks" in _sys.modules:
        return
    try:
        from trn_agent_boot.trn_boot import _ntff_profile_via_ctypes

        hook = _ntff_profile_via_ctypes("/opt/axon/libaxon_pjrt.so")
    except Exception:
        hook = None
    mod = types.ModuleType("antenv.axon_hooks")
    _state = {"hook": hook}
    mod.set_axon_ntff_profile_hook = lambda h: _state.__setitem__("hook", h)
    mod.get_axon_ntff_profile_hook = lambda: _state["hook"]
    _sys.modules["antenv.axon_hooks"] = mod


def run_cores(meta: Meta, trace: bool = False):
    from concourse.bass_utils import run_bass_kernel_spmd

    if trace:
        _install_ntff_hook_shim()

    nc = build_program(meta.dims)
    GSZ = GPB * P
    iota512 = np.tile(np.arange(GSZ, dtype=np.int16)[None, :], (P, 1))
    identf = np.eye(P, dtype=np.float32)
    in_maps = []
    for c in range(NCORES):
        in_maps.append(
            {
                "x16": meta.x16,
                "srcidx": meta.srcidx[c],
                "ndv": meta.ndv[c],
                "mg": meta.mg[c],
                "mgt": meta.mgt[c],
                "invcnt": meta.invcnt[c],
                "w16": meta.w16,
                "bias16": meta.bias16,
                "alpha_gp": meta.alpha_gp,
                "gnw_gp": meta.gnw_gp,
                "gnb_gp": meta.gnb_gp,
                "iota512": iota512,
                "identf": identf,
            }
        )
    res = run_bass_kernel_spmd(nc, in_maps, core_ids=list(range(NCORES)), trace=trace)
    return res


def unshard(meta: Meta, results):
    dims = meta.dims
    N = meta.node_perm.shape[0]
    h = np.empty((N, P), np.float32)
    flat = np.full((NUM_GRAPHS, P), -np.inf, np.float32)
    node_pos = 0
    for c in range(NCORES):
        h_c = results[c]["h_out"]
        bmax_c = results[c]["bmax_out"]  # [128, NB]
        for g, cnt, ps in meta.core_graphs[c]:
            if cnt == 0:
                continue
            h[node_pos : node_pos + cnt] = h_c[ps : ps + cnt]
            node_pos += cnt
            b0, b1 = ps // P, (ps + cnt - 1) // P + 1
            flat[g] = bmax_c[:, b0:b1].max(axis=1)
    assert node_pos == N
    return h, flat


def kernel(inputs, edge_index, batch, edge_weight, W, b, gn_weight, gn_bias, gn_alpha):
    meta = preprocess(
        inputs, edge_index, batch, edge_weight, W, b, gn_weight, gn_bias, gn_alpha
    )
    res = run_cores(meta)
    return unshard(meta, res.results)


# ---------------------------------------------------------------------------
# numpy emulation of the device program (debugging aid; mirrors the Bass code)
def emulate_core(meta: Meta, c: int):
    d = meta.dims
    NB, CPQ, GP, K, NCH = d.NB, d.CPQ, d.GP, d.K, d.NCH
    GSZ = GPB * P
    x16 = meta.x16
    iota = np.arange(GSZ, dtype=np.float32)
    out_all = np.zeros((NB, P, P), np.float32)
    sums = np.zeros((GP, P), np.float32)
    sumsq = np.zeros((GP, P), np.float32)
    for grp in range(d.NGRP):
        aggT = np.zeros((P, GSZ), np.float32)
        for j in range(K):
            q, cq = j // CPQ, j % CPQ
            colbase = (grp * 4 + q) * CPQ * 8
            jj = cq * P + np.arange(P)
            i16 = meta.srcidx[c, jj % 16, colbase + jj // 16].astype(np.int64)
            src_rows = i16 * 4 + q
            xgc = x16[src_rows].astype(np.float32)  # [128 lanes, 128 feat]
            jg = grp * K + j
            dloc = meta.ndv[c, :, jg : jg + 1]
            nv = meta.ndv[c, :, NCH + jg : NCH + jg + 1]
            oh = ((iota[None, :] == dloc) * nv).astype(NPF16).astype(np.float32)
            aggT += xgc.T @ oh
        aggT = aggT.astype(NPF16).astype(np.float32)
        for bb in range(GPB):
            b_ = grp * GPB + bb
            out = aggT[:, bb * P : (bb + 1) * P].T @ meta.w16.astype(
                np.float32
            ) + meta.bias16.astype(np.float32)
            out16 = out.astype(NPF16).astype(np.float32)
            out_all[b_] = out16
            mgb = meta.mg[c, :, b_ * GP : (b_ + 1) * GP].astype(np.float32)
            sums += mgb.T @ out16
            sumsq += mgb.T @ (out16**2).astype(NPF16).astype(np.float32)
    mu = sums * meta.invcnt[c]
    esq = sumsq * meta.invcnt[c]
    am = mu * meta.alpha_gp
    var = esq - (2 * mu - am) * am
    rstd = 1.0 / np.sqrt(var + EPS)
    rg = rstd * meta.gnw_gp
    q_t = meta.gnb_gp - am * rg
    combo = np.concatenate([rg, q_t], axis=1).astype(NPF16).astype(np.float32)
    h_core = np.zeros((NB * P, P), np.float32)
    bmax = np.zeros((P, NB), np.float32)
    for b_ in range(NB):
        mgtb = meta.mgt[c, :, b_ * P : (b_ + 1) * P].astype(np.float32)
        rqb = mgtb.T @ combo  # [128, 256]
        h = out_all[b_] * rqb[:, :P] + rqb[:, P:]
        h = np.maximum(h, 0.0)
        h_core[b_ * P : (b_ + 1) * P] = h
        bmax[:, b_] = h.T.max(axis=1)
    return {"h_out": h_core, "bmax_out": bmax}


def kernel_emulated(inputs, edge_index, batch, edge_weight, W, b, gn_weight, gn_bias, gn_alpha):
    meta = preprocess(
        inputs, edge_index, batch, edge_weight, W, b, gn_weight, gn_bias, gn_alpha
    )
    results = [emulate_core(meta, c) for c in range(NCORES)]
    return unshard(meta, results)


# revision 24
# speedup vs baseline: 1.3841x; 1.3841x over previous
"""GCN block (GCNConv + GraphNorm + ReLU + global_max_pool) on 8 Trainium2 cores.

Strategy (matches the "assign whole graphs to devices" sharding hint):
- Host: partition the 64 graphs into 8 contiguous node-balanced groups, one per
  core.  Within a core, each graph's nodes are padded to a multiple of 128 so
  every 128-node block belongs to exactly one graph.  Edges (plus self-loops)
  are bucketed by destination block and cut into 128-edge chunks.  All of this
  is index-only preprocessing; feature compute happens on device.
- Device (one SPMD Bass program on 8 NeuronCores, no collectives):
    per block: indirect-DMA gather of source rows (fp16), build the scatter
    one-hot with one fused DVE tensor_scalar (iota == dloc) * norm, accumulate
    aggT = sum_chunks XgT @ OneHot on the PE in PSUM, then out = aggT.T @ W + b,
    GraphNorm statistics via node->graph one-hot matmuls accumulated in PSUM,
    then h = out * rstd*gamma + (beta - alpha*mean*rstd*gamma), ReLU,
    PE-transpose + reduce_max per block for the pooling, DMA h out.
- Host: unshard h; fold per-block maxes into per-graph flat.
"""

import sys

if "/opt/trn_rl_repo" not in sys.path:
    sys.path.insert(0, "/opt/trn_rl_repo")

from contextlib import ExitStack
from dataclasses import dataclass

import numpy as np

import concourse.bacc as bacc
import concourse.bass as bass
import concourse.tile as tile
from concourse import mybir
from concourse._compat import get_trn_type
from concourse.library_config import mlp

F16 = mybir.dt.bfloat16
NPF16 = mybir.dt.np(mybir.dt.bfloat16)
F32 = mybir.dt.float32
I16 = mybir.dt.int16
I32 = mybir.dt.int32

NCORES = 8
P = 128
GPB = 4  # blocks per indirect-gather group
EPS = 1e-5
NUM_GRAPHS = 64


@dataclass
class Dims:
    n_rows: int  # rows of the (shared) fp16 node-feature table
    NB: int  # 128-node blocks per core
    CPQ: int  # 128-edge chunks per (group, parity class)
    GP: int  # graph slots per core (last slot = pad graph)

    @property
    def NGRP(self):
        return self.NB // GPB

    @property
    def K(self):
        # chunk slots per group: 4 parity classes x CPQ
        return 4 * self.CPQ

    @property
    def NCH(self):
        return self.NGRP * self.K


@dataclass
class Meta:
    dims: Dims
    x16: np.ndarray  # [n_rows, 128] f16
    srcidx: np.ndarray  # [NCORES, 128, NGRP*K*8] i16 gather indices (wrap-16, rows 0:32)
    ndv: np.ndarray  # [NCORES, 128, 2*NCH] f32: dloc cols [0:NCH], norm cols [NCH:2NCH]
    mg: np.ndarray  # [NCORES, 128, NB*GP] f16
    mgt: np.ndarray  # [NCORES, GP, NB*128] f16
    invcnt: np.ndarray  # [NCORES, GP, 1] f32
    w16: np.ndarray  # [128, 128] f16
    bias16: np.ndarray  # [1, 128] f16
    alpha_gp: np.ndarray  # [GP, 128] f32
    gnw_gp: np.ndarray  # [GP, 128] f32
    gnb_gp: np.ndarray  # [GP, 128] f32
    # unshard info
    core_graphs: list  # per core: list of (global_graph, count, pad_start_slot)
    node_perm: np.ndarray  # [N] global slot index per node (core-major)


def _split_graphs(pblocks: np.ndarray, ncores: int) -> list[int]:
    """Split graphs (given per-graph block counts) into ncores contiguous parts
    minimizing the max blocks per part. Returns boundaries s[0..ncores]."""
    G = len(pblocks)
    cum = np.concatenate([[0], np.cumsum(pblocks)])

    def feasible(B):
        s, parts = 0, 0
        while s < G:
            e = int(np.searchsorted(cum, cum[s] + B, side="right")) - 1
            if e <= s:
                return None
            parts += 1
            s = e
            if parts > ncores:
                return None
        return parts

    lo, hi = int(pblocks.max()), int(cum[-1])
    while lo < hi:
        mid = (lo + hi) // 2
        if feasible(mid):
            hi = mid
        else:
            lo = mid + 1
    B = lo
    bounds = [0]
    s = 0
    for c in range(ncores):
        remaining = ncores - c - 1
        e = int(np.searchsorted(cum, cum[s] + B, side="right")) - 1
        e = max(e, s + 1) if s < G else s
        # don't leave more graphs than remaining cores can take (trivially fine)
        e = min(e, G - 0)
        if remaining == 0:
            e = G
        bounds.append(e)
        s = e
    assert bounds[-1] == G, bounds
    return bounds


def preprocess(inputs, edge_index, batch, edge_weight, W, b, gn_weight, gn_bias, gn_alpha) -> Meta:
    inputs = np.asarray(inputs)
    N = inputs.shape[0]
    row = np.asarray(edge_index[0], dtype=np.int64)
    col = np.asarray(edge_index[1], dtype=np.int64)
    w = np.asarray(edge_weight, dtype=np.float32)
    batch = np.asarray(batch, dtype=np.int64)
    G = NUM_GRAPHS

    # --- GCN symmetric norm (with self-loops), f32 like the reference ---
    deg = np.bincount(col, weights=w.astype(np.float64), minlength=N).astype(np.float32)
    deg += 1.0  # self-loop weight
    dis = (1.0 / np.sqrt(deg)).astype(np.float32)
    norm_e = (dis[row] * w * dis[col]).astype(np.float32)
    norm_self = (dis * dis).astype(np.float32)

    # --- graph partitioning: contiguous groups of whole graphs ---
    counts = np.bincount(batch, minlength=G).astype(np.int64)
    gstart = np.concatenate([[0], np.cumsum(counts)])  # node range per graph
    pblocks = (counts + P - 1) // P  # blocks per graph (0 for empty)
    bounds = _split_graphs(np.maximum(pblocks, 0), NCORES)

    NB = 0
    core_graphs = []
    core_of_graph = np.zeros(G, np.int64)
    for c in range(NCORES):
        gs, ge = bounds[c], bounds[c + 1]
        core_of_graph[gs:ge] = c
        lst = []
        off = 0
        for g in range(gs, ge):
            lst.append((g, int(counts[g]), off * P))
            off += int(pblocks[g])
        core_graphs.append(lst)
        NB = max(NB, off)
    NB = ((NB + GPB - 1) // GPB) * GPB
    GP = max(len(lst) for lst in core_graphs) + 1

    # global node -> (core, local slot)
    slot_of_node = np.empty(N, np.int64)
    core_of_node = core_of_graph[batch]
    for c in range(NCORES):
        for g, cnt, ps in core_graphs[c]:
            s0 = gstart[g]
            slot_of_node[s0 : s0 + cnt] = ps + np.arange(cnt)

    # --- per-core edge buckets ---
    # combine real edges and self-loops
    all_src = np.concatenate([row, np.arange(N, dtype=np.int64)])
    all_dst = np.concatenate([col, np.arange(N, dtype=np.int64)])
    all_val = np.concatenate([norm_e, norm_self])
    dst_core = core_of_node[all_dst]
    dst_slot = slot_of_node[all_dst]

    # bucket edges by (group of 4 blocks, src parity class); sort by (grp, q)
    GSZ = GPB * P  # 512 dests per group
    NGRP = NB // GPB
    CPQ = 1
    per_core = []
    for c in range(NCORES):
        m = dst_core == c
        dsl = dst_slot[m]
        srcs = all_src[m]
        vals = all_val[m]
        grp = dsl // GSZ
        q = (srcs & 3).astype(np.int64)
        key = grp * 4 + q
        order = np.argsort(key, kind="stable")
        dsl, srcs, vals, key = dsl[order], srcs[order], vals[order], key[order]
        cnt_key = np.bincount(key, minlength=NGRP * 4)
        starts = np.concatenate([[0], np.cumsum(cnt_key)])
        pos = np.arange(len(dsl)) - starts[key]
        CPQ = max(CPQ, int((pos >> 7).max()) + 1)
        per_core.append((dsl, srcs, vals, key, pos))

    dims = Dims(n_rows=N, NB=NB, CPQ=CPQ, GP=GP)
    K = dims.K
    NCH = dims.NCH

    srcidx = np.zeros((NCORES, P, dims.NGRP * K * 8), np.int16)
    ndv = np.zeros((NCORES, P, 2 * NCH), np.float32)
    mg = np.zeros((NCORES, P, NB * GP), NPF16)
    mgt = np.zeros((NCORES, GP, NB * P), NPF16)
    invcnt = np.ones((NCORES, GP, 1), np.float32)

    for c in range(NCORES):
        dsl, srcs, vals, key, pos = per_core[c]
        grp, q = key // 4, key % 4
        cq = pos >> 7  # chunk within (grp, q)
        lane = pos & 127
        dloc = (dsl % GSZ).astype(np.float32)
        # global chunk slot
        jg = grp * K + q * CPQ + cq
        ndv[c, lane, jg] = dloc
        ndv[c, lane, NCH + jg] = vals
        # gather index: slot j within the (grp, q) gather = cq*128 + lane,
        # wrapped to [j % 16, j // 16]; HW (queue 0) reads partitions 16..31,
        # CoreSim reads 0..15 -> fill both.
        j = cq * P + lane
        colbase = (grp * 4 + q) * CPQ * 8
        i16v = (srcs >> 2).astype(np.int16)
        for m in range(5):
            srcidx[c, 16 * m + (j % 16), colbase + (j // 16)] = i16v

        # node-slot -> graph-slot map
        gsl = np.full(NB * P, GP - 1, np.int64)
        for gl, (g, cnt, ps) in enumerate(core_graphs[c]):
            gsl[ps : ps + cnt] = gl
            invcnt[c, gl, 0] = 1.0 / max(cnt, 1)
        slots = np.arange(NB * P)
        mg[c, slots & 127, (slots >> 7) * GP + gsl] = 1.0
        # mgt: pad-node columns stay all-zero so h_pad = relu(0*out + 0) = 0,
        # which never exceeds a real (relu'd, >= 0) per-graph max.
        real = gsl < GP - 1
        mgt[c, gsl[real], slots[real]] = 1.0
        # pad-graph slot: zero stats (its h/flat are discarded; keeps var >= 0)
        invcnt[c, GP - 1, 0] = 0.0

    alpha = np.asarray(gn_alpha, np.float32).reshape(1, P)
    gnw = np.asarray(gn_weight, np.float32).reshape(1, P)
    gnb = np.asarray(gn_bias, np.float32).reshape(1, P)

    return Meta(
        dims=dims,
        x16=np.ascontiguousarray(inputs.astype(NPF16)),
        srcidx=srcidx,
        ndv=ndv,
        mg=mg,
        mgt=mgt,
        invcnt=invcnt,
        w16=np.asarray(W, np.float32).astype(NPF16),
        bias16=np.asarray(b, np.float32).reshape(1, P).astype(NPF16),
        alpha_gp=np.repeat(alpha, GP, axis=0),
        gnw_gp=np.repeat(gnw, GP, axis=0),
        gnb_gp=np.repeat(gnb, GP, axis=0),
        core_graphs=core_graphs,
        node_perm=slot_of_node,
    )


def _legalize_waits(nc: bass.Bass) -> None:
    """This toolchain's walrus encodes at most ONE sync wait per instruction;
    Tile can emit several.  (1) Drop waits that same-engine program order
    already guarantees (Tile is not transitively minimal); (2) hoist remaining
    extras into standalone InstEventSemaphore instructions just before the
    consumer on the same engine."""
    updaters: dict[int, set] = {}
    for f in nc.m.functions:
        for bb in f.blocks:
            for ins in bb.instructions:
                si = ins.sync_info
                if not si or not si.on_update:
                    continue
                is_async = "DMA" in type(ins).__name__ or "Collective" in type(ins).__name__
                for u in si.on_update:
                    updaters.setdefault(u.id, set()).add((ins.engine, is_async))

    counter = 0
    for f in nc.m.functions:
        for bb in f.blocks:
            snapshot = list(bb.instructions)
            insert_at = 0
            live = bb.instructions
            for ins in snapshot:
                idx = None
                si = ins.sync_info
                if si and si.on_wait and len(si.on_wait) > 1:
                    keep = []
                    for w in si.on_wait:
                        ups = updaters.get(w.id)
                        if (
                            ups
                            and ins.engine != mybir.EngineType.PE
                            and all(
                                eng == ins.engine and not a for eng, a in ups
                            )
                        ):
                            continue  # same-engine in-order execution covers it
                        keep.append(w)
                    if not keep:
                        keep = [si.on_wait[-1]]
                    extras, keep = keep[:-1], keep[-1:]
                    while si.on_wait:
                        si.on_wait.pop()
                    si.on_wait.append(keep[0])
                    if extras:
                        idx = next(
                            i for i, x in enumerate(live) if x.name == ins.name
                        )
                        for w in extras:
                            counter += 1
                            ev = mybir.InstEventSemaphore(
                                name=f"I-lglw-{counter}", ins=[], outs=[]
                            )
                            ev.engine = ins.engine
                            evsi = ev.sync_info
                            if evsi is None:
                                ev.sync_info = mybir.SyncInfo(on_wait=[w], on_update=[])
                            else:
                                evsi.on_wait.append(w)
                            nc.register_instruction(ev, overwrite=True)
                            live.insert(idx, ev)
                            idx += 1


def build_program(dims: Dims) -> bass.Bass:
    NB, CPQ, GP, K, NGRP = dims.NB, dims.CPQ, dims.GP, dims.K, dims.NGRP
    NCH = dims.NCH
    GSZ = GPB * P
    nc = bacc.Bacc(get_trn_type() or "TRN2", num_swdge_queues=2)

    x16 = nc.dram_tensor("x16", [dims.n_rows, P], F16, kind="ExternalInput")
    srcidx = nc.dram_tensor("srcidx", [P, NGRP * K * 8], I16, kind="ExternalInput")
    ndv = nc.dram_tensor("ndv", [P, 2 * NCH], F32, kind="ExternalInput")
    mg = nc.dram_tensor("mg", [P, NB * GP], F16, kind="ExternalInput")
    mgt = nc.dram_tensor("mgt", [GP, NB * P], F16, kind="ExternalInput")
    invcnt = nc.dram_tensor("invcnt", [GP, 1], F32, kind="ExternalInput")
    w16 = nc.dram_tensor("w16", [P, P], F16, kind="ExternalInput")
    bias16 = nc.dram_tensor("bias16", [1, P], F16, kind="ExternalInput")
    alpha_gp = nc.dram_tensor("alpha_gp", [GP, P], F32, kind="ExternalInput")
    gnw_gp = nc.dram_tensor("gnw_gp", [GP, P], F32, kind="ExternalInput")
    gnb_gp = nc.dram_tensor("gnb_gp", [GP, P], F32, kind="ExternalInput")
    iota512 = nc.dram_tensor("iota512", [P, GSZ], I16, kind="ExternalInput")
    identf = nc.dram_tensor("identf", [P, P], F32, kind="ExternalInput")

    h_out = nc.dram_tensor("h_out", [NB * P, P], F32, kind="ExternalOutput")
    bmax_out = nc.dram_tensor("bmax_out", [P, NB], F32, kind="ExternalOutput")

    # strided parity views of the node table: class q = rows q, q+4, q+8, ...
    xviews = []
    for q in range(4):
        ap = bass.AP(tensor=x16, offset=q * P, ap=[[4 * P, dims.n_rows // 4], [1, P]])
        xviews.append(ap)

    with tile.TileContext(nc) as tc, ExitStack() as ctx:
        const = ctx.enter_context(tc.tile_pool(name="const", bufs=1))
        meta = ctx.enter_context(tc.tile_pool(name="meta", bufs=1))
        gpool = ctx.enter_context(tc.tile_pool(name="gather", bufs=3))
        ohp = ctx.enter_context(tc.tile_pool(name="onehot", bufs=4))
        work = ctx.enter_context(tc.tile_pool(name="work", bufs=3))
        outall = ctx.enter_context(tc.tile_pool(name="outall", bufs=NB))
        hpool = ctx.enter_context(tc.tile_pool(name="hpool", bufs=3))
        psA = ctx.enter_context(tc.tile_pool(name="psA", bufs=2, space="PSUM"))
        psO = ctx.enter_context(tc.tile_pool(name="psO", bufs=2, space="PSUM"))
        psS = ctx.enter_context(tc.tile_pool(name="psS", bufs=1, space="PSUM"))
        psB = ctx.enter_context(tc.tile_pool(name="psB", bufs=1, space="PSUM"))
        psT = ctx.enter_context(tc.tile_pool(name="psT", bufs=1, space="PSUM"))

        # gpsimd runs ONLY the library load + dma_gathers (custom ucode safety)
        nc.gpsimd.load_library(mlp)

        # ---- constants (all host-shipped; no gpsimd compute) ----
        iota_sb = const.tile([P, GSZ], I16)
        nc.sync.dma_start(iota_sb[:], iota512[:])
        ident = const.tile([P, P], F32)
        nc.sync.dma_start(ident[:], identf[:])
        ones1 = const.tile([1, P], F16)
        nc.vector.memset(ones1[:], 1.0)
        eps_t = const.tile([GP, 1], F32)
        nc.vector.memset(eps_t[:], EPS)

        w_sb = const.tile([P, P], F16)
        nc.sync.dma_start(w_sb[:], w16[:])
        b_sb = const.tile([1, P], F16)
        nc.sync.dma_start(b_sb[:], bias16[:])
        invc_sb = const.tile([GP, 1], F32)
        nc.sync.dma_start(invc_sb[:], invcnt[:])
        alpha_sb = const.tile([GP, P], F32)
        nc.sync.dma_start(alpha_sb[:], alpha_gp[:])
        gnw_sb = const.tile([GP, P], F32)
        nc.sync.dma_start(gnw_sb[:], gnw_gp[:])
        gnb_sb = const.tile([GP, P], F32)
        nc.sync.dma_start(gnb_sb[:], gnb_gp[:])

        # ---- resident metadata ----
        srcidx_sb = meta.tile([P, NGRP * K * 8], I16)
        nc.sync.dma_start(srcidx_sb[:], srcidx[:])
        nd_sb = meta.tile([P, 2 * NCH], F32)
        nc.sync.dma_start(nd_sb[:], ndv[:])
        mg_sb = meta.tile([P, NB * GP], F16)
        nc.sync.dma_start(mg_sb[:], mg[:])
        mgt_sb = meta.tile([GP, NB * P], F16)
        nc.sync.dma_start(mgt_sb[:], mgt[:])

        # ---- persistent accumulators ----
        sums_t = psS.tile([GP, P], F32, tag="sums")
        sumsq_t = psS.tile([GP, P], F32, tag="sumsq")
        sums_ps = sums_t[:]
        sumsq_ps = sumsq_t[:]
        bmax_sb = meta.tile([P, NB], F32, tag="bmax")

        out_tiles = []

        # ---- phase A/B: gather + aggregate + linear + stats ----
        for grp in range(NGRP):
            xg = gpool.tile([P, K, P], F16)
            for q in range(4):
                colbase = (grp * 4 + q) * CPQ * 8
                nc.gpsimd.dma_gather(
                    out_ap=xg[:, q * CPQ : (q + 1) * CPQ, :],
                    in_ap=xviews[q],
                    idxs_ap=srcidx_sb[:, colbase : colbase + CPQ * 8],
                    num_idxs=CPQ * P,
                    num_idxs_reg=CPQ * P,
                    elem_size=P,
                    elem_step=4 * P,
                    queue_num=q // 2,
                )
            aggT = psA.tile([P, GSZ], F32)
            for j in range(K):
                jg = grp * K + j
                oh = ohp.tile([P, GSZ], F16, tag="oh")
                nc.vector.tensor_scalar(
                    out=oh[:],
                    in0=iota_sb[:],
                    scalar1=nd_sb[:, jg : jg + 1],
                    scalar2=nd_sb[:, NCH + jg : NCH + jg + 1],
                    op0=mybir.AluOpType.is_equal,
                    op1=mybir.AluOpType.mult,
                )
                nc.tensor.matmul(
                    out=aggT[:],
                    lhsT=xg[:, j, :],
                    rhs=oh[:],
                    start=(j == 0),
                    stop=(j == K - 1),
                )
            aggT_sb = work.tile([P, GSZ], F16, tag="aggT")
            nc.scalar.copy(aggT_sb[:], aggT[:])
            for bb in range(GPB):
                b_ = grp * GPB + bb
                outp = psO.tile([P, P], F32)
                nc.tensor.matmul(
                    outp[:], lhsT=aggT_sb[:, bb * P : (bb + 1) * P], rhs=w_sb[:],
                    start=True, stop=False,
                )
                nc.tensor.matmul(outp[:], lhsT=ones1[:], rhs=b_sb[:], start=False, stop=True)
                out_sb = outall.tile([P, P], F16, tag="out")
                nc.scalar.copy(out_sb[:], outp[:])
                out_tiles.append(out_sb)
                sq_sb = work.tile([P, P], F16, tag="sq")
                nc.scalar.square(sq_sb[:], outp[:])
                nc.tensor.matmul(
                    out=sums_ps,
                    lhsT=mg_sb[:, b_ * GP : (b_ + 1) * GP],
                    rhs=out_sb[:],
                    start=(b_ == 0),
                    stop=(b_ == NB - 1),
                )
                nc.tensor.matmul(
                    out=sumsq_ps,
                    lhsT=mg_sb[:, b_ * GP : (b_ + 1) * GP],
                    rhs=sq_sb[:],
                    start=(b_ == 0),
                    stop=(b_ == NB - 1),
                )

        # ---- phase C: finalize GraphNorm stats ([GP, 128] tiles) ----
        mu = work.tile([GP, P], F32, tag="mu")
        nc.vector.tensor_scalar(
            out=mu[:], in0=sums_ps, scalar1=invc_sb[:, 0:1], scalar2=None,
            op0=mybir.AluOpType.mult,
        )
        esq = work.tile([GP, P], F32, tag="esq")
        nc.vector.tensor_scalar(
            out=esq[:], in0=sumsq_ps, scalar1=invc_sb[:, 0:1], scalar2=None,
            op0=mybir.AluOpType.mult,
        )
        am = work.tile([GP, P], F32, tag="am")
        nc.vector.tensor_tensor(out=am[:], in0=mu[:], in1=alpha_sb[:], op=mybir.AluOpType.mult)
        t2 = work.tile([GP, P], F32, tag="t2")
        nc.vector.tensor_scalar(
            out=t2[:], in0=mu[:], scalar1=2.0, scalar2=None, op0=mybir.AluOpType.mult
        )
        nc.vector.tensor_tensor(out=t2[:], in0=t2[:], in1=am[:], op=mybir.AluOpType.subtract)
        nc.vector.tensor_tensor(out=t2[:], in0=t2[:], in1=am[:], op=mybir.AluOpType.mult)
        var = work.tile([GP, P], F32, tag="var")
        nc.vector.tensor_tensor(out=var[:], in0=esq[:], in1=t2[:], op=mybir.AluOpType.subtract)
        sd = work.tile([GP, P], F32, tag="sd")
        nc.scalar.activation(
            out=sd[:], in_=var[:], func=mybir.ActivationFunctionType.Sqrt,
            bias=eps_t[:, 0:1], scale=1.0,
        )
        rstd = work.tile([GP, P], F32, tag="rstd")
        nc.vector.reciprocal(out=rstd[:], in_=sd[:])
        rg = work.tile([GP, P], F32, tag="rg")
        nc.vector.tensor_tensor(out=rg[:], in0=rstd[:], in1=gnw_sb[:], op=mybir.AluOpType.mult)
        q_t = work.tile([GP, P], F32, tag="q_t")
        nc.vector.tensor_tensor(out=q_t[:], in0=am[:], in1=rg[:], op=mybir.AluOpType.mult)
        nc.vector.tensor_tensor(out=q_t[:], in0=gnb_sb[:], in1=q_t[:], op=mybir.AluOpType.subtract)
        combo = const.tile([GP, 2 * P], F16)
        nc.scalar.copy(combo[:, 0:P], rg[:])
        nc.scalar.copy(combo[:, P : 2 * P], q_t[:])

        # ---- phase D: normalize + relu + write + block max ----
        for b_ in range(NB):
            rq = psB.tile([P, 2 * P], F32, tag="rq")
            nc.tensor.matmul(
                out=rq[:],
                lhsT=mgt_sb[:, b_ * P : (b_ + 1) * P],
                rhs=combo[:],
                start=True,
                stop=True,
            )
            hpre = hpool.tile([P, P], F32, tag="hpre")
            nc.vector.tensor_tensor(
                out=hpre[:], in0=out_tiles[b_][:], in1=rq[:, 0:P], op=mybir.AluOpType.mult
            )
            nc.vector.tensor_tensor(
                out=hpre[:], in0=hpre[:], in1=rq[:, P : 2 * P], op=mybir.AluOpType.add
            )
            hsb = hpool.tile([P, P], F32, tag="hsb")
            nc.scalar.activation(
                out=hsb[:], in_=hpre[:], func=mybir.ActivationFunctionType.Relu
            )
            nc.sync.dma_start(h_out[b_ * P : (b_ + 1) * P, :], hsb[:])
            htp = psT.tile([P, P], F32, tag="htp")
            nc.tensor.transpose(htp[:], hsb[:], ident[:])
            nc.vector.reduce_max(bmax_sb[:, b_ : b_ + 1], htp[:], axis=mybir.AxisListType.X)

        nc.sync.dma_start(bmax_out[:], bmax_sb[:])

    _legalize_waits(nc)
    nc.compile()
    return nc


def _install_ntff_hook_shim():
    """The agent image's ``antenv`` lacks ``axon_hooks``; recreate it so
    ``run_bass_kernel_spmd(trace=True)`` can capture NTFF profiles."""
    import sys as _sys
    import types

    if "antenv.axon_hooks" in _sys.modules:
        return
    try:
        from trn_agent_boot.trn_boot import _ntff_profile_via_ctypes

        hook = _ntff_profile_via_ctypes("/opt/axon/libaxon_pjrt.so")
    except Exception:
        hook = None
    mod = types.ModuleType("antenv.axon_hooks")
    _state = {"hook": hook}
    mod.set_axon_ntff_profile_hook = lambda h: _state.__setitem__("hook", h)
    mod.get_axon_ntff_profile_hook = lambda: _state["hook"]
    _sys.modules["antenv.axon_hooks"] = mod


def run_cores(meta: Meta, trace: bool = False):
    from concourse.bass_utils import run_bass_kernel_spmd

    if trace:
        _install_ntff_hook_shim()

    nc = build_program(meta.dims)
    GSZ = GPB * P
    iota512 = np.tile(np.arange(GSZ, dtype=np.int16)[None, :], (P, 1))
    identf = np.eye(P, dtype=np.float32)
    in_maps = []
    for c in range(NCORES):
        in_maps.append(
            {
                "x16": meta.x16,
                "srcidx": meta.srcidx[c],
                "ndv": meta.ndv[c],
                "mg": meta.mg[c],
                "mgt": meta.mgt[c],
                "invcnt": meta.invcnt[c],
                "w16": meta.w16,
                "bias16": meta.bias16,
                "alpha_gp": meta.alpha_gp,
                "gnw_gp": meta.gnw_gp,
                "gnb_gp": meta.gnb_gp,
                "iota512": iota512,
                "identf": identf,
            }
        )
    res = run_bass_kernel_spmd(nc, in_maps, core_ids=list(range(NCORES)), trace=trace)
    return res


def unshard(meta: Meta, results):
    dims = meta.dims
    N = meta.node_perm.shape[0]
    h = np.empty((N, P), np.float32)
    flat = np.full((NUM_GRAPHS, P), -np.inf, np.float32)
    node_pos = 0
    for c in range(NCORES):
        h_c = results[c]["h_out"]
        bmax_c = results[c]["bmax_out"]  # [128, NB]
        for g, cnt, ps in meta.core_graphs[c]:
            if cnt == 0:
                continue
            h[node_pos : node_pos + cnt] = h_c[ps : ps + cnt]
            node_pos += cnt
            b0, b1 = ps // P, (ps + cnt - 1) // P + 1
            flat[g] = bmax_c[:, b0:b1].max(axis=1)
    assert node_pos == N
    return h, flat


def kernel(inputs, edge_index, batch, edge_weight, W, b, gn_weight, gn_bias, gn_alpha):
    meta = preprocess(
        inputs, edge_index, batch, edge_weight, W, b, gn_weight, gn_bias, gn_alpha
    )
    res = run_cores(meta)
    return unshard(meta, res.results)


# ---------------------------------------------------------------------------
# numpy emulation of the device program (debugging aid; mirrors the Bass code)
def emulate_core(meta: Meta, c: int):
    d = meta.dims
    NB, CPQ, GP, K, NCH = d.NB, d.CPQ, d.GP, d.K, d.NCH
    GSZ = GPB * P
    x16 = meta.x16
    iota = np.arange(GSZ, dtype=np.float32)
    out_all = np.zeros((NB, P, P), np.float32)
    sums = np.zeros((GP, P), np.float32)
    sumsq = np.zeros((GP, P), np.float32)
    for grp in range(d.NGRP):
        aggT = np.zeros((P, GSZ), np.float32)
        for j in range(K):
            q, cq = j // CPQ, j % CPQ
            colbase = (grp * 4 + q) * CPQ * 8
            jj = cq * P + np.arange(P)
            i16 = meta.srcidx[c, jj % 16, colbase + jj // 16].astype(np.int64)
            src_rows = i16 * 4 + q
            xgc = x16[src_rows].astype(np.float32)  # [128 lanes, 128 feat]
            jg = grp * K + j
            dloc = meta.ndv[c, :, jg : jg + 1]
            nv = meta.ndv[c, :, NCH + jg : NCH + jg + 1]
            oh = ((iota[None, :] == dloc) * nv).astype(NPF16).astype(np.float32)
            aggT += xgc.T @ oh
        aggT = aggT.astype(NPF16).astype(np.float32)
        for bb in range(GPB):
            b_ = grp * GPB + bb
            out = aggT[:, bb * P : (bb + 1) * P].T @ meta.w16.astype(
                np.float32
            ) + meta.bias16.astype(np.float32)
            out16 = out.astype(NPF16).astype(np.float32)
            out_all[b_] = out16
            mgb = meta.mg[c, :, b_ * GP : (b_ + 1) * GP].astype(np.float32)
            sums += mgb.T @ out16
            sumsq += mgb.T @ (out16**2).astype(NPF16).astype(np.float32)
    mu = sums * meta.invcnt[c]
    esq = sumsq * meta.invcnt[c]
    am = mu * meta.alpha_gp
    var = esq - (2 * mu - am) * am
    rstd = 1.0 / np.sqrt(var + EPS)
    rg = rstd * meta.gnw_gp
    q_t = meta.gnb_gp - am * rg
    combo = np.concatenate([rg, q_t], axis=1).astype(NPF16).astype(np.float32)
    h_core = np.zeros((NB * P, P), np.float32)
    bmax = np.zeros((P, NB), np.float32)
    for b_ in range(NB):
        mgtb = meta.mgt[c, :, b_ * P : (b_ + 1) * P].astype(np.float32)
        rqb = mgtb.T @ combo  # [128, 256]
        h = out_all[b_] * rqb[:, :P] + rqb[:, P:]
        h = np.maximum(h, 0.0)
        h_core[b_ * P : (b_ + 1) * P] = h
        bmax[:, b_] = h.T.max(axis=1)
    return {"h_out": h_core, "bmax_out": bmax}


def kernel_emulated(inputs, edge_index, batch, edge_weight, W, b, gn_weight, gn_bias, gn_alpha):
    meta = preprocess(
        inputs, edge_index, batch, edge_weight, W, b, gn_weight, gn_bias, gn_alpha
    )
    results = [emulate_core(meta, c) for c in range(NCORES)]
    return unshard(meta, results)
